# revision 42
# baseline (speedup 1.0000x reference)
"""Trainium2 Bass kernel for GroupedQuerySelfAttention (head-TP sharding).

Problem: B=2, N=2048, D=2048, H=8 kv-heads, G=4 (32 query heads), C=64.
  q = (x @ Wq) / sqrt(32);  kv = x @ Wkv;  k, v = split(kv)
  per (b, h, g): S = Qg K^T;  A = softmax(S);  O = A V
  out = concat_heads(O) @ Wp + bp

Sharding: 8 cores = 2 batches x 4 kv-head-pairs (tensor parallel over the
kv-head dim, per the sharding hint). Each core owns 2 kv heads and their
8 (h, g) query heads over the full sequence: it projects only its slice
of Q/K/V (no duplicated projection work), runs attention for its 8 pairs,
and computes a partial output projection over its 512 o-columns. The
all-reduce after the output projection is folded into the host gather
(partials are summed on host; bias added there too).

Per-core layouts (everything bf16 except psum):
  XT  [128,4,16,512]  x^T via transposing DMA (host pre-casts x to bf16)
  QT  [128,4,2048]  Q^T: pair (g, l) lives at rows l*64+c, block g
  KT  [128,2048]    K^T: local head l at rows l*64+c
  VT  [128,16,2,65] V rows [n, c] per (sb, l) + ones column (softmax denom)
  S^T psum [128 s, <=1024 q] <- 512-row matmuls; one wide exp per psum
  PV  psum [128 q, 4, 65] accumulated over sb: O rows + denominator col
  OT  [128,4,2048]  o^T via PE transpose of normalized O blocks
  out [2048, 2048] bf16 partial = o^T.T @ Wp (host sums 4 partials + bias)

The emission order hand-interleaves projection work into the gaps of the
ACT-bound exp stream (ACT is the second-busiest engine at ~267us; PE
~325us) and starts attention on the first 512 query columns as soon as
K-chunk 0 and Q-chunk 0 are projected.
"""

import numpy as np
from contextlib import ExitStack

import concourse.bass as bass
import concourse.tile as tile
from concourse import bacc, mybir
from concourse.bass_utils import run_bass_kernel_spmd
from concourse.masks import make_identity

P = 128
B, N, D = 2, 2048, 2048
H, G, C = 8, 4, 64
DB = D // P                    # 16 d-blocks
NB = N // P                    # 16 seq blocks
SCALE = float(1.0 / np.sqrt(H * G))
F32 = mybir.dt.float32
BF16 = mybir.dt.bfloat16
AF = mybir.ActivationFunctionType


def build_program(n_cores=8):
    nc = bacc.Bacc("TRN2", target_bir_lowering=False, debug=False,
                   num_devices=n_cores)
    xb = nc.dram_tensor("xb", [N, D], BF16, kind="ExternalInput").ap()
    wq = nc.dram_tensor("wq", [DB, P, 512], BF16, kind="ExternalInput").ap()
    wkv = nc.dram_tensor("wkv", [DB, P, 256], BF16,
                         kind="ExternalInput").ap()
    wp = nc.dram_tensor("wp", [4, P, D], BF16, kind="ExternalInput").ap()
    out = nc.dram_tensor("out", [N, D], BF16, kind="ExternalOutput").ap()

    with tile.TileContext(nc) as tc, ExitStack() as top:
        # ---- persistent stores ----
        store = top.enter_context(tc.tile_pool(name="store", bufs=1))
        QT = store.tile([P, 4, N], BF16, tag="QT")
        KT = store.tile([P, N], BF16, tag="KT")
        VT = store.tile([P, NB, 2, C + 1], BF16, tag="VT")
        # WQ dies (last Q projection) before OT is born (first o^T
        # transpose), so they share one slot via the same tag
        wqot = top.enter_context(tc.tile_pool(name="wqot", bufs=1))
        WQ = wqot.tile([P, DB, 512], BF16, tag="wqot")
        WKV = store.tile([P, DB, 256], BF16, tag="WKV")
        WP = store.tile([P, 4, N], BF16, tag="WP")
        identB = store.tile([P, P], BF16, tag="identB")
        make_identity(nc, identB[:])
        nc.gpsimd.memset(VT[:, :, :, C:C + 1], 1.0)

        # weight loads lead the two hw DGE queues (wkv is needed first);
        # x^T is built by the transposing DMA, split across both queues
        # chunk-major so chunk 0 lands first; wp rides the gpsimd path
        XT_pool = top.enter_context(tc.tile_pool(name="XT", bufs=3))
        xbf_pool = top.enter_context(
            tc.tile_pool(name="xbf", bufs=4, side="right"))
        xt_tiles = {}

        def xt_slot(ch):
            if ch not in xt_tiles:
                xt_tiles[ch] = XT_pool.tile([P, DB, 512], BF16, tag="XT",
                                            name=f"xt{ch}")
            return xt_tiles[ch]

        xbf_tiles = {}

        def load_rows(ch):
            """chunks 0/1: plain row loads (cheap on the serial DMA path)"""
            for rt in range(4):
                r0 = ch * 512 + rt * P
                xr = xbf_pool.tile([P, D], BF16, tag="xbf",
                                   name=f"xr{ch}_{rt}")
                eng = nc.sync if rt % 2 == 0 else nc.scalar
                eng.dma_start(xr[:], xb[r0:r0 + P, :])
                xbf_tiles[(ch, rt)] = xr

        # persistent psum pool: one bank-sized f32 tag shared by the
        # K/Q/V-projection accumulators and the output-projection
        # accumulators (2 banks)
        bank = top.enter_context(tc.tile_pool(name="bank", bufs=2,
                                              space="PSUM"))

        def phase_K(ch):
            xt = xt_slot(ch)
            kps = bank.tile([P, 512], F32, tag="bank", name=f"kps{ch}")
            for db in range(DB):
                nc.tensor.matmul(kps[:], WKV[:, db, 0:P], xt[:, db, :],
                                 start=(db == 0), stop=(db == DB - 1))
            nc.vector.tensor_copy(KT[:, ch * 512:(ch + 1) * 512], kps[:])

        def phase_V(ch):
            xt = xt_slot(ch)
            vps = bank.tile([P, 4, 2, C], F32, tag="bank", name=f"vps{ch}")
            for nb4 in range(4):
                for db in range(DB):
                    nc.tensor.matmul(vps[:, nb4, :, :],
                                     xt[:, db, nb4 * P:(nb4 + 1) * P],
                                     WKV[:, db, P:2 * P],
                                     start=(db == 0), stop=(db == DB - 1))
            for nb4 in range(4):
                sb = ch * 4 + nb4
                nc.vector.tensor_copy(VT[:, sb, :, 0:C], vps[:, nb4, :, :])

        def phase_Q(ch, gs=range(4)):
            xt = xt_slot(ch)
            for g in gs:
                qps = bank.tile([P, 512], F32, tag="bank",
                                name=f"qps{ch}_{g}")
                for db in range(DB):
                    nc.tensor.matmul(qps[:], WQ[:, db, g * P:(g + 1) * P],
                                     xt[:, db, :],
                                     start=(db == 0), stop=(db == DB - 1))
                nc.vector.tensor_copy(QT[:, g, ch * 512:(ch + 1) * 512],
                                      qps[:])

        # ---- attention + output projection, software-pipelined ----
        cps = top.enter_context(
            tc.tile_pool(name="cps", bufs=1, space="PSUM"))
        csb = top.enter_context(tc.tile_pool(name="csb", bufs=2))

        def trans_rows(ch, rts=range(4)):
            """PE-transpose row tiles of a chunk into its XT slot"""
            xt = xt_slot(ch)
            for rt in rts:
                xr = xbf_tiles.pop((ch, rt))
                for dq in range(4):
                    tp = cps.tile([P, 4, P], BF16, tag="qk", bufs=2,
                                  name=f"tp{ch}_{rt}_{dq}")
                    for i in range(4):
                        nc.tensor.transpose(
                            tp[:, i, :],
                            xr[:, (dq * 4 + i) * P:(dq * 4 + i + 1) * P],
                            identB[:])
                    nc.vector.tensor_copy(
                        xt[:, dq * 4:dq * 4 + 4, rt * P:(rt + 1) * P],
                        tp[:])
        ost_tiles = {}
        e_tiles = {}
        ot_holder = {}

        def get_OT():
            if "OT" not in ot_holder:
                ot_holder["OT"] = wqot.tile([P, 4, N], BF16, tag="wqot",
                                            name="OT")
            return ot_holder["OT"]

        def emit_qk(qh, p, sb_list, cols=(0, 1024)):
            g, l = p // 2, p % 2
            key = (qh, p)
            if key not in e_tiles:
                e_tiles[key] = csb.tile([P, NB, 1024], BF16, tag="E",
                                        name=f"E{qh}_{p}")
            E = e_tiles[key]
            o = l * C
            c0, c1 = cols
            for sb in sb_list:
                qk = cps.tile([P, c1 - c0], F32, tag="qk", bufs=2,
                              name=f"qk{qh}_{p}_{sb}_{c0}")
                for h2 in range((c1 - c0) // 512):
                    nc.tensor.matmul(
                        qk[:, h2 * 512:(h2 + 1) * 512],
                        KT[o:o + C, sb * P:(sb + 1) * P],
                        QT[o:o + C, g, qh * 1024 + c0 + h2 * 512:
                           qh * 1024 + c0 + (h2 + 1) * 512],
                        start=True, stop=True)
                nc.scalar.activation(E[:, sb, c0:c1], qk[:], AF.Exp,
                                     scale=SCALE)

        def emit_pv(qh, p, halves=range(2)):
            g, l = p // 2, p % 2
            E = e_tiles[(qh, p)]
            if qh not in ost_tiles:
                ost_tiles[qh] = csb.tile([P, 8, 8, C], BF16, tag="Ost",
                                         bufs=1, name=f"Ost{qh}")
            Ost = ost_tiles[qh]
            for h4 in halves:
                pv = cps.tile([P, 4, C + 1], F32, tag="pv", bufs=1,
                              name=f"pv{qh}_{p}_{h4}")
                for qi in range(4):
                    qb = h4 * 4 + qi
                    for sb in range(NB):
                        nc.tensor.matmul(pv[:, qi, :],
                                         E[:, sb, qb * P:(qb + 1) * P],
                                         VT[:, sb, l, :],
                                         start=(sb == 0),
                                         stop=(sb == NB - 1))
                rec = csb.tile([P, 4], F32, tag="rec", bufs=2,
                               name=f"rec{qh}_{p}_{h4}")
                nc.vector.reciprocal(rec[:], pv[:, :, C])
                nc.vector.tensor_mul(
                    Ost[:, h4 * 4:h4 * 4 + 4, p, :], pv[:, :, 0:C],
                    rec[:, :, None].to_broadcast((P, 4, C)))

        def emit_ot_trans(qh, qbs=range(8)):
            Ost = ost_tiles[qh]
            OT = get_OT()
            for qb in qbs:
                tp2 = cps.tile([P, 4, P], BF16, tag="tp2", bufs=1,
                               name=f"tp2_{qh}_{qb}")
                for g in range(4):
                    nc.tensor.transpose(tp2[:, g, :],
                                        Ost[:, qb, 2 * g:2 * g + 2, :],
                                        identB[:])
                for g in range(4):
                    nc.vector.tensor_copy(
                        OT[:, g, qh * 1024 + qb * P:qh * 1024 + (qb + 1) * P],
                        tp2[:, g, :])

        def emit_outproj(qh, qb):
            OT = get_OT()
            qw = qh * 1024 + qb * P
            for half in range(2):
                osb = csb.tile([P, 1024], BF16, tag="osb", bufs=3,
                               name=f"osb{qh}_{qb}_{half}")
                for dc2 in range(2):
                    dc = half * 2 + dc2
                    ops = bank.tile([P, 512], F32, tag="bank",
                                    name=f"ops{qh}_{qb}_{dc}")
                    for jb in range(4):
                        nc.tensor.matmul(ops[:], OT[:, jb, qw:qw + P],
                                         WP[:, jb, dc * 512:(dc + 1) * 512],
                                         start=(jb == 0), stop=(jb == 3))
                    nc.vector.tensor_copy(osb[:, dc2 * 512:(dc2 + 1) * 512],
                                          ops[:])
                eng = nc.sync if half == 0 else nc.scalar
                eng.dma_start(
                    out[qw:qw + P, half * 1024:(half + 1) * 1024], osb[:])

        # ---- main interleaved emission ----
        # preamble DMAs: wkv first (K0 needs it), chunk 0+1 rows, then wq
        # in two halves behind them on the scalar queue (the DMA transfer
        # path is serial in the model, so order matters)
        with tc.high_priority():
            nc.sync.dma_start(WKV[:], wkv[:, :, :].transpose([1, 0, 2]))
            load_rows(0)
            load_rows(1)
            for wh in range(2):
                nc.sync.dma_start(WQ[:, :, wh * 256:(wh + 1) * 256],
                                  wq[:, :, wh * 256:(wh + 1) * 256]
                                  .transpose([1, 0, 2]))
            nc.gpsimd.dma_start(WP[:], wp[:, :, :].transpose([1, 0, 2]))

        warm = cps.tile([P, P], BF16, tag="tp2", bufs=1, name="warm")
        for i in range(150):
            nc.tensor.transpose(warm[:], identB[:], identB[:])
        trans_rows(0)
        phase_K(0)
        trans_rows(1)
        phase_K(1)
        # attention starts on query cols 0:512 (pairs 0/1 need only the
        # g0 slice of Q-chunk 0); the rest of the projections thread into
        # the exp-stream gaps
        phase_Q(0, [0])
        emit_qk(0, 0, range(0, 4), (0, 512))
        phase_V(0)
        emit_qk(0, 0, range(4, 8), (0, 512))
        phase_Q(0, [1])
        emit_qk(0, 1, range(0, 8), (0, 512))
        phase_Q(1, [0])
        emit_qk(0, 0, range(0, 4), (512, 1024))
        phase_Q(0, [2])
        emit_qk(0, 0, range(4, 8), (512, 1024))
        phase_Q(0, [3])
        emit_qk(0, 1, range(0, 4), (512, 1024))
        load_rows(2)
        trans_rows(2)
        phase_V(1)
        phase_Q(1, [1])
        emit_qk(0, 1, range(4, 8), (512, 1024))
        load_rows(3)
        trans_rows(3)
        phase_K(2)
        emit_qk(0, 0, range(8, 12))
        phase_K(3)
        emit_qk(0, 1, range(8, 12))
        phase_V(2)
        emit_qk(0, 0, range(12, 16))
        phase_V(3)
        emit_qk(0, 1, range(12, 16))
        emit_pv(0, 0)

        # filler units consumed between QK batches of qh0 pairs 2..7;
        # ordered so each Q slice lands just before the pair that needs it
        fillers = [
            lambda: phase_Q(1, [2]), lambda: phase_Q(1, [3]),
            lambda: phase_Q(2, [0]), lambda: phase_Q(3, [0]),
            lambda: phase_Q(2, [1]), lambda: phase_Q(3, [1]),
            lambda: phase_Q(2, [2]), lambda: phase_Q(3, [2]),
            lambda: phase_Q(2, [3]), lambda: phase_Q(3, [3]),
        ]  # Q-chunk slices land just before the pairs that need them
        for p in range(2, 8):
            emit_qk(0, p, range(0, 8))
            emit_pv(0, p - 1, [0])
            if fillers:
                fillers.pop(0)()
            emit_qk(0, p, range(8, 16))
            emit_pv(0, p - 1, [1])
            if fillers:
                fillers.pop(0)()

        # qh1 pairs with qh0's transposes + output projection as filler
        for p in range(8):
            if p == 7:
                # column-split the last pair so its first PV group can
                # start before the second half of its exps finishes
                emit_qk(1, 7, range(0, 16), (0, 512))
                emit_outproj(0, 6)
                emit_pv(1, 6)
                emit_qk(1, 7, range(0, 16), (512, 1024))
                emit_outproj(0, 7)
                continue
            emit_qk(1, p, range(0, 8))
            if p == 0:
                emit_pv(0, 7)
                emit_qk(1, 0, range(8, 16))
                emit_ot_trans(0)
                continue
            if p < 7:
                emit_pv(1, p - 1, [0])
            emit_qk(1, p, range(8, 16))
            if p < 7:
                emit_pv(1, p - 1, [1])
            emit_outproj(0, p - 1)

        # tail: last pair's PV, o^T transposes and output projection are
        # pipelined per 4-qb group to keep PE streaming to the end
        emit_pv(1, 7, [0])
        emit_ot_trans(1, range(0, 4))
        emit_outproj(1, 0)
        emit_pv(1, 7, [1])
        emit_outproj(1, 1)
        emit_ot_trans(1, range(4, 6))
        emit_outproj(1, 2)
        emit_ot_trans(1, range(6, 8))
        emit_outproj(1, 3)
        for qb in range(4, 8):
            emit_outproj(1, qb)

    nc.compile()
    return nc


_nc_cache = None

# query-head column order per core: j_local = g*128 + l*64 + c maps to
# original column (2*hp + l)*G*C + g*C + c  (same permutation for Wq cols
# and Wp rows)
def _perm(hp):
    idx = np.empty(512, np.int64)
    for g in range(G):
        for l in range(2):
            base = (2 * hp + l) * G * C + g * C
            idx[g * 128 + l * 64:g * 128 + l * 64 + 64] = np.arange(
                base, base + C)
    return idx


def kernel(x, Wq, Wkv, Wp, bp):
    global _nc_cache
    if _nc_cache is None:
        _nc_cache = build_program()
    nc = _nc_cache
    import ml_dtypes
    x = np.asarray(x, dtype=np.float32)
    xb16 = [np.ascontiguousarray(x[b]).astype(ml_dtypes.bfloat16)
            for b in range(B)]
    Wq = np.asarray(Wq, dtype=np.float32)
    Wkv = np.asarray(Wkv, dtype=np.float32)
    Wp = np.asarray(Wp, dtype=np.float32)
    bp = np.asarray(bp, dtype=np.float32)

    in_maps = []
    for c in range(8):
        b, hp = c // 4, c % 4
        idx = _perm(hp)
        wq_c = np.ascontiguousarray(Wq[:, idx]).astype(
            ml_dtypes.bfloat16).reshape(DB, P, 512)
        wkv_c = np.ascontiguousarray(np.concatenate(
            [Wkv[:, hp * P:(hp + 1) * P],
             Wkv[:, H * C + hp * P:H * C + (hp + 1) * P]],
            axis=1)).astype(ml_dtypes.bfloat16).reshape(DB, P, 256)
        wp_c = np.ascontiguousarray(Wp[idx, :]).astype(
            ml_dtypes.bfloat16).reshape(4, P, D)
        in_maps.append({"xb": xb16[b], "wq": wq_c, "wkv": wkv_c,
                        "wp": wp_c})
    res = run_bass_kernel_spmd(nc, in_maps, list(range(8)))
    outp = np.empty((B, N, D), np.float32)
    for b in range(B):
        acc = np.zeros((N, D), np.float32)
        for hp in range(4):
            acc += np.asarray(res.results[b * 4 + hp]["out"],
                              dtype=np.float32)
        outp[b] = acc + bp
    return outp
